# revision 2
# baseline (speedup 1.0000x reference)
"""Trainium2 Bass kernel for nn_OADModel: Linear+ReLU -> LSTM(512 steps) -> Linear.

Sharding: data-parallel over batch across 8 NeuronCores (8 samples/core).
v2: Phase 3 uses PE column tiling in bf16. A TRN2 matmul may target PSUM
partition bases {0,32,64,96} (col groups; bf16 only — fp32r's PE mode is
incompatible), so the four hidden-quarters' gate matmuls run CONCURRENTLY
in the four 32-column groups of the array, quadrupling effective PE
throughput for this batch-8 recurrence. Gates land at psum rows 32q, so
the eltwise cell update runs as a few [128,*] ACT/DVE instructions; the
final h=o*tanh(c) products shift partitions back to base 0 (DVE operands
may have per-operand partition bases) so the hT transposes keep the
baseline's base-0 form.

Per core (B=8 local samples, T=512 steps, tokens ordered t-major: col = t*8+b):
  P1: xT[1024, 4096] = relu(W_pre @ fT + b_pre)        (PE, fp32r)
  P2: gx[4096tok, 4096g] = xT.T @ W_ihT + (b_ih+b_hh)  (PE, fp32r; output
      rounded to bf16; gate axis host-permuted to [i_q f_q g_q o_q] per
      256-wide hidden quarter q)
  P3: 512-step LSTM recurrence (col-tiled bf16, see above)
  P4: scoresT[22, 4096] = W_cls @ hT_hist + b_cls      (bf16, batched at end)
Host reassembles [B*T, 22] from the 8 scoresT outputs.
"""
import sys

for _p in ("/opt/trn_rl_repo",):
    if _p not in sys.path:
        sys.path.insert(0, _p)

import numpy as np
import concourse.bass as bass
import concourse.mybir as mybir
import concourse.tile as tile
from concourse import bacc
from concourse.bass_utils import run_bass_kernel_spmd
from concourse.masks import make_identity

FP = mybir.dt.float32
FR = mybir.dt.float32r
BF = mybir.dt.bfloat16
AF = mybir.ActivationFunctionType

F = 3072      # feature size
LI = 1024     # LSTM input
H = 1024      # LSTM hidden
G = 4096      # 4*H gates
C = 22        # classes
B_FULL = 64
NCORES = 8
BL = B_FULL // NCORES        # 8 samples per core
P = 128
KF = F // P   # 24 k-tiles over feature
KI = LI // P  # 8 k-tiles over LSTM input
KH = H // P   # 8 k-tiles over hidden
HQ = H // 4   # 256 hidden per quarter
HIST = 16     # recurrence steps per history tile


def build_kernel(T=512, t0=0, t1=None, debug_outputs=False):
    """One program covering recurrence steps [t0, t1) of T total.

    The first program (t0==0) also runs phases 1-2 and OUTPUTS gx; later
    programs take gx plus the carried h/c state as inputs. Each program
    emits scores for its own step window."""
    if t1 is None:
        t1 = T
    TOK = BL * T
    first = (t0 == 0)
    last = (t1 == T)
    n_tok_tiles = TOK // P          # stationary token tiles in P2
    n_chunks = TOK // 512           # 512-token chunks in P1
    nc = bacc.Bacc(None, target_bir_lowering=False)

    if first:
        fT = nc.dram_tensor("fT", [F, TOK], FR, kind="ExternalInput")
        WpT = nc.dram_tensor("WpT", [F, LI], FR, kind="ExternalInput")
        bp = nc.dram_tensor("bp", [LI], FP, kind="ExternalInput")
        WiT = nc.dram_tensor("WiT", [LI, G], FR, kind="ExternalInput")
        bg = nc.dram_tensor("bg", [G], FP, kind="ExternalInput")
        gx_h = nc.dram_tensor("gx_h", [TOK, G], BF, kind="ExternalOutput")
    else:
        gx_h = nc.dram_tensor("gx_h", [TOK, G], BF, kind="ExternalInput")
        h0 = nc.dram_tensor("h0", [H, BL], BF, kind="ExternalInput")
        c0 = nc.dram_tensor("c0", [BL, H], FP, kind="ExternalInput")
    WhT = nc.dram_tensor("WhT", [H, G], BF, kind="ExternalInput")
    WcT = nc.dram_tensor("WcT", [H, C], BF, kind="ExternalInput")
    bc = nc.dram_tensor("bc", [C, 1], FP, kind="ExternalInput")
    if first:
        xT_h = nc.dram_tensor("xT_h", [LI, TOK], FR)
    hT_h = nc.dram_tensor("hT_h", [H, TOK], BF)
    sT = nc.dram_tensor("sT", [C, (t1 - t0) * BL], FP, kind="ExternalOutput")
    if not last:
        h_end = nc.dram_tensor("h_end", [H, BL], BF, kind="ExternalOutput")
        c_end = nc.dram_tensor("c_end", [BL, H], FP, kind="ExternalOutput")

    with tile.TileContext(nc) as tc:
        if first:
            # ---------------- Phase 1: xT = relu(WpT.T-tiles @ fT + bp) -------
            with (
                tc.tile_pool(name="p1w", bufs=1) as p1w,
                tc.tile_pool(name="p1f", bufs=26) as p1f,
                tc.tile_pool(name="p1o", bufs=4) as p1o,
                tc.tile_pool(name="p1b", bufs=1) as p1b,
                tc.tile_pool(name="p1ps", bufs=6, space="PSUM") as p1ps,
            ):
                wp_sb = p1w.tile([P, KF, LI], FR)
                nc.sync.dma_start(wp_sb[:], WpT[:].rearrange("(k p) m -> p k m", p=P))
                bp_sb = p1b.tile([P, KI], FP)
                nc.sync.dma_start(bp_sb[:], bp[:].rearrange("(m p) -> p m", p=P))
                for cch in range(n_chunks):
                    fts = []
                    for k in range(KF):
                        ft = p1f.tile([P, 512], FR, tag="ft")
                        nc.sync.dma_start(
                            ft[:], fT[k * P:(k + 1) * P, cch * 512:(cch + 1) * 512])
                        fts.append(ft)
                    for m in range(KI):
                        ps = p1ps.tile([P, 512], FP)
                        for k in range(KF):
                            nc.tensor.matmul(
                                ps[:], wp_sb[:, k, m * P:(m + 1) * P], fts[k][:],
                                start=(k == 0), stop=(k == KF - 1))
                        xo = p1o.tile([P, 512], FR, tag="xo")
                        nc.scalar.activation(xo[:], ps[:], AF.Relu,
                                             bias=bp_sb[:, m:m + 1])
                        nc.sync.dma_start(
                            xT_h[m * P:(m + 1) * P, cch * 512:(cch + 1) * 512], xo[:])

            # ---------------- Phase 2: gx = xT.T @ WiT + bg -------------------
            with (
                tc.tile_pool(name="p2w", bufs=1) as p2w,
                tc.tile_pool(name="p2x", bufs=3) as p2x,
                tc.tile_pool(name="p2o", bufs=4) as p2o,
                tc.tile_pool(name="p2b", bufs=1) as p2b,
                tc.tile_pool(name="p2ps", bufs=1, space="PSUM") as p2ps,
            ):
                wi_sb = p2w.tile([P, KI, G], FR)
                nc.sync.dma_start(wi_sb[:], WiT[:].rearrange("(k p) g -> p k g", p=P))
                bg_bc = p2b.tile([P, G], FP)
                nc.sync.dma_start(
                    bg_bc[:], bass.AP(tensor=bg, offset=0, ap=[[0, P], [1, G]]))
                for m in range(n_tok_tiles):
                    xs = p2x.tile([P, KI, P], FR, tag="xs")
                    nc.sync.dma_start(
                        xs[:], xT_h[:, m * P:(m + 1) * P].rearrange(
                            "(k p) n -> p k n", p=P))
                    pss = [p2ps.tile([P, 512], FP, tag=f"gps{gch}", name=f"gps{gch}_{m}") for gch in range(8)]
                    for k in range(KI):
                        for gch in range(8):
                            nc.tensor.matmul(
                                pss[gch][:], xs[:, k, :],
                                wi_sb[:, k, gch * 512:(gch + 1) * 512],
                                start=(k == 0), stop=(k == KI - 1))
                    for gch in range(8):
                        go = p2o.tile([P, 512], BF, tag="go")
                        nc.vector.tensor_tensor(
                            go[:], pss[gch][:], bg_bc[:, gch * 512:(gch + 1) * 512],
                            op=mybir.AluOpType.add)
                        nc.sync.dma_start(
                            gx_h[m * P:(m + 1) * P, gch * 512:(gch + 1) * 512], go[:])

        # ---------------- Phase 3: LSTM recurrence (col-tiled bf16) -------
        # Gate layout per step (host gate-perm gives [i_q f_q g_q o_q] per
        # 256-wide hidden quarter q at offset q*1024):
        #   P_A[32q:32q+8, 0:512]  <- chunk A_q = [i_q | f_q]
        #   P_B[32q:32q+8, 0:512]  <- chunk B_q = [g_q | o_q]
        # c lives in [128, 256] tiles: quarter q at partitions 32q:32q+8;
        # h is shifted back to [BL, H] at base 0 for the transposes.
        with (
            tc.tile_pool(name="p3w", bufs=1) as p3w,
            tc.tile_pool(name="p3gx", bufs=1) as p3gx,
            tc.tile_pool(name="p3hist", bufs=2) as p3hist,
            tc.tile_pool(name="p3st", bufs=3) as p3st,
            tc.tile_pool(name="p3tmp", bufs=2) as p3tmp,
            tc.tile_pool(name="p3one", bufs=1) as p3one,
            tc.tile_pool(name="p3pg", bufs=2, space="PSUM") as p3pg,
            tc.tile_pool(name="p3pt", bufs=2, space="PSUM") as p3pt,
        ):
            wh_sb = p3w.tile([P, KH, G], BF)
            nc.sync.dma_start(wh_sb[:], WhT[:].rearrange("(k p) g -> p k g", p=P))

            # identities: i128 (rows 0:BL diag, bf16) for the gx psum-init
            # matmul; i8f (fp32) for the base-0 PE transposes
            i128f = p3one.tile([P, BL], FP)
            nc.gpsimd.memset(i128f[:], 0.0)
            make_identity(nc, i128f[0:BL, 0:BL], nomemset=True)
            i128 = p3one.tile([P, BL], BF)
            nc.vector.tensor_copy(i128[:], i128f[:])
            i8f = p3one.tile([BL, BL], FP)
            make_identity(nc, i8f[:])
            i8b = p3one.tile([BL, BL], BF)
            nc.vector.tensor_copy(i8b[:], i8f[:])

            if first:
                z8f = p3one.tile([P, BL], FP)
                nc.vector.memset(z8f[:], 0.0)
                z8 = p3one.tile([P, BL], BF)
                nc.vector.tensor_copy(z8[:], z8f[:])
                zc = p3one.tile([P, HQ], FP)
                nc.vector.memset(zc[:], 0.0)
                c_init = zc
            else:
                h0_sb = p3one.tile([P, KH, BL], BF)
                nc.sync.dma_start(h0_sb[:], h0[:].rearrange("(k p) b -> p k b", p=P))
                c0_sb = p3one.tile([P, HQ], FP)
                for q in range(4):
                    nc.sync.dma_start(c0_sb[32 * q:32 * q + BL, :],
                                      c0[:, q * HQ:(q + 1) * HQ])
                c_init = c0_sb

            # two manually-rotated gx staging buffers; rows BL:128 are only
            # read against the zero rows of the i128 stationary, but memset
            # them once so nothing uninitialized feeds the PE
            gxbufs = []
            for i in range(2):
                gxb = p3gx.tile([P, G], BF, name=f"gxp{i}")
                nc.vector.memset(gxb[:], 0.0)
                gxbufs.append(gxb)

            hist_tiles = {}
            c_prev = c_init
            for t in range(t0, t1):
                s = t % HIST
                if s == 0 or t == t0:
                    hist = p3hist.tile([P, KH, HIST, BL], BF, tag="hist",
                                       name=f"hist_{t}")
                    hist_tiles[t // HIST] = hist

                def h_prev(k):
                    if t == t0:
                        return z8[:, 0:BL] if first else h0_sb[:, k, :]
                    ps_, pk_ = divmod(t - 1, HIST)
                    return hist_tiles[ps_][:, k, pk_, :]

                gxp = gxbufs[t % 2]
                nc.sync.dma_start(gxp[0:BL, :], gx_h[t * BL:(t + 1) * BL, :])

                pa = p3pg.tile([P, 512], FP, tag="pa")
                pb = p3pg.tile([P, 512], FP, tag="pb")
                # psum-init with gx (independent of h -> can run early)
                for q in range(4):
                    nc.tensor.matmul(pa[32 * q:32 * q + BL, :], i128[:],
                                     gxp[:, q * 1024:q * 1024 + 512],
                                     start=True, stop=False,
                                     tile_position=(0, 32 * q))
                    nc.tensor.matmul(pb[32 * q:32 * q + BL, :], i128[:],
                                     gxp[:, q * 1024 + 512:(q + 1) * 1024],
                                     start=True, stop=False,
                                     tile_position=(0, 32 * q))
                # recurrent term: 4 col groups advance together per k
                for k in range(KH):
                    for q in range(4):
                        nc.tensor.matmul(pa[32 * q:32 * q + BL, :], h_prev(k),
                                         wh_sb[:, k, q * 1024:q * 1024 + 512],
                                         start=False, stop=(k == KH - 1),
                                         tile_position=(0, 32 * q))
                        nc.tensor.matmul(pb[32 * q:32 * q + BL, :], h_prev(k),
                                         wh_sb[:, k, q * 1024 + 512:(q + 1) * 1024],
                                         start=False, stop=(k == KH - 1),
                                         tile_position=(0, 32 * q))

                # eltwise cell update, all 4 quarters per instruction; the
                # c path stays fp32, the h path drops to bf16 (2x DVE rate)
                sa = p3tmp.tile([P, 512], FP, tag="sa")    # [sig i | sig f]
                tg = p3tmp.tile([P, 512], BF, tag="tg")    # [tanh g | sig o]
                ig = p3tmp.tile([P, HQ], FP, tag="ig")
                th = p3tmp.tile([P, HQ], BF, tag="th")
                c_new = p3st.tile([P, HQ], FP, tag="c")
                h_sb = p3st.tile([BL, H], BF, tag="h")
                nc.scalar.activation(sa[:], pa[:], AF.Sigmoid)
                nc.scalar.activation(tg[:, 0:HQ], pb[:, 0:HQ], AF.Tanh)
                nc.scalar.activation(tg[:, HQ:512], pb[:, HQ:512], AF.Sigmoid)
                nc.vector.tensor_tensor(ig[:], sa[:, 0:HQ], tg[:, 0:HQ],
                                        op=mybir.AluOpType.mult)
                nc.vector.tensor_tensor(c_new[:], sa[:, HQ:512], c_prev[:],
                                        op=mybir.AluOpType.mult)
                nc.vector.tensor_tensor(c_new[:], c_new[:], ig[:],
                                        op=mybir.AluOpType.add)
                nc.scalar.activation(th[:], c_new[:], AF.Tanh)
                c_prev = c_new

                # per quarter: h = o*tanh(c) shifted to base 0, its two hT
                # transposes, and its history copy — so the k=2q,2q+1
                # matmuls of the next step unblock before later quarters
                # finish (they are issued in k order)
                pt = p3pt.tile([P, KH * BL], BF, tag="pt")
                for q in range(4):
                    nc.vector.tensor_tensor(
                        h_sb[:, q * HQ:(q + 1) * HQ],
                        tg[32 * q:32 * q + BL, HQ:512],
                        th[32 * q:32 * q + BL, :],
                        op=mybir.AluOpType.mult)
                    for k in (2 * q, 2 * q + 1):
                        nc.tensor.transpose(pt[:, k * BL:(k + 1) * BL],
                                            h_sb[:, k * P:(k + 1) * P], i8b[:])
                    nc.scalar.activation(
                        hist[:, 2 * q:2 * q + 2, s, :],
                        pt[:, 16 * q:16 * q + 16].rearrange(
                            "p (k b) -> p k b", b=BL),
                        AF.Identity)

                if s == HIST - 1:
                    tb = t - (HIST - 1)
                    for k in range(KH):
                        nc.sync.dma_start(
                            hT_h[k * P:(k + 1) * P, tb * BL:(tb + HIST) * BL],
                            hist[:, k, :, :])
                if t == t1 - 1 and not last:
                    for k in range(KH):
                        nc.sync.dma_start(h_end[k * P:(k + 1) * P, :],
                                          hist[:, k, s, :])
                    for q in range(4):
                        nc.sync.dma_start(c_end[:, q * HQ:(q + 1) * HQ],
                                          c_new[32 * q:32 * q + BL, :])

        # ---------------- Phase 4: scoresT = WcT.T-tiles @ hT + bc --------
        with (
            tc.tile_pool(name="p4w", bufs=1) as p4w,
            tc.tile_pool(name="p4h", bufs=3) as p4h,
            tc.tile_pool(name="p4o", bufs=3) as p4o,
            tc.tile_pool(name="p4ps", bufs=4, space="PSUM") as p4ps,
        ):
            wc_sb = p4w.tile([P, KH, C], BF)
            nc.sync.dma_start(wc_sb[:], WcT[:].rearrange("(k p) c -> p k c", p=P))
            bc_sb = p4w.tile([C, 1], FP)
            nc.sync.dma_start(bc_sb[:], bc[:])
            for cch in range(t0 * BL // 512, t1 * BL // 512):
                hs = p4h.tile([P, KH, 512], BF, tag="hs")
                nc.sync.dma_start(
                    hs[:], hT_h[:, cch * 512:(cch + 1) * 512].rearrange(
                        "(k p) n -> p k n", p=P))
                ps = p4ps.tile([C, 512], FP)
                for k in range(KH):
                    nc.tensor.matmul(ps[:], wc_sb[:, k, :], hs[:, k, :],
                                     start=(k == 0), stop=(k == KH - 1))
                so = p4o.tile([C, 512], FP, tag="so")
                nc.scalar.activation(so[:], ps[:], AF.Identity, bias=bc_sb[:])
                nc.sync.dma_start(
                    sT[:, cch * 512 - t0 * BL:(cch + 1) * 512 - t0 * BL], so[:])

    nc.compile()
    return nc


_NC_CACHE = {}


def _get_nc(T, t0, t1):
    key = (T, t0, t1)
    if key not in _NC_CACHE:
        _NC_CACHE[key] = build_kernel(T, t0, t1)
    return _NC_CACHE[key]


def _gate_perm():
    """Permutation of the 4096 gate axis: [i f g o] blocks of 1024 ->
    per 256-wide hidden quarter q: [i_q f_q g_q o_q]."""
    idx = []
    for q in range(4):
        for gate in range(4):
            s = gate * H + q * 256
            idx.extend(range(s, s + 256))
    return np.array(idx)


def kernel(feature_in, W_pre, b_pre, W_ih, b_ih, W_hh, b_hh, W_cls, b_cls,
           T_override=None, trace=False, n_splits=2):
    import ml_dtypes
    bf16 = ml_dtypes.bfloat16
    T = int(feature_in.shape[1]) if T_override is None else T_override
    TOK = BL * T
    perm = _gate_perm()

    WpT = np.ascontiguousarray(W_pre.T.astype(np.float32))
    WiT = np.ascontiguousarray(W_ih.T[:, perm].astype(np.float32))
    bgv = (b_ih + b_hh).astype(np.float32)[perm]
    WhT = np.ascontiguousarray(W_hh.T[:, perm].astype(bf16))
    WcT = np.ascontiguousarray(W_cls.T.astype(bf16))
    bcv = b_cls.astype(np.float32).reshape(C, 1)

    bounds = [T * i // n_splits for i in range(n_splits + 1)]
    total_ns = 0
    traces = []
    score_parts = [[] for _ in range(NCORES)]   # per core, per split
    carry = None                                # (h_end, c_end) per core
    for si in range(n_splits):
        t0, t1 = bounds[si], bounds[si + 1]
        nc = _get_nc(T, t0, t1)
        in_maps = []
        for core in range(NCORES):
            m = dict(WhT=WhT, WcT=WcT, bc=bcv)
            if t0 == 0:
                fs = feature_in[core * BL:(core + 1) * BL, :T]
                m["fT"] = np.ascontiguousarray(
                    fs.transpose(2, 1, 0).reshape(F, TOK).astype(np.float32))
                m.update(WpT=WpT, bp=b_pre.astype(np.float32), WiT=WiT, bg=bgv)
            else:
                m["gx_h"] = carry_gx[core]
                m["h0"] = carry[core][0]
                m["c0"] = carry[core][1]
            in_maps.append(m)
        res = run_bass_kernel_spmd(nc, in_maps, core_ids=list(range(NCORES)),
                                   trace=trace)
        if res.exec_time_ns is not None:
            total_ns += res.exec_time_ns
        if res.instructions_and_trace:
            traces.append(res.instructions_and_trace[1])
        for core in range(NCORES):
            r = res.results[core]
            sc = r["sT"]                                       # [22, (t1-t0)*BL]
            score_parts[core].append(sc.reshape(C, t1 - t0, BL))
        if t1 < T:
            if t0 == 0:
                carry_gx = [res.results[core]["gx_h"] for core in range(NCORES)]
            carry = [(res.results[core]["h_end"], res.results[core]["c_end"])
                     for core in range(NCORES)]

    outs = []
    for core in range(NCORES):
        sc = np.concatenate(score_parts[core], axis=1)         # [22, T, 8]
        outs.append(sc.transpose(2, 1, 0))                     # [8, T, 22]
    full = np.concatenate(outs, axis=0)                        # [64, T, 22]
    out = full.reshape(B_FULL * T, C).astype(np.float32)
    kernel.last_exec_time_ns = total_ns if total_ns else None
    kernel.last_trace = traces
    return out


kernel.last_exec_time_ns = None
kernel.last_trace = None


# revision 3
# speedup vs baseline: 1.0959x; 1.0959x over previous
"""Trainium2 Bass kernel for nn_OADModel: Linear+ReLU -> LSTM(512 steps) -> Linear.

Sharding: data-parallel over batch across 8 NeuronCores (8 samples/core).
v2: Phase 3 uses PE column tiling in bf16. A TRN2 matmul may target PSUM
partition bases {0,32,64,96} (col groups; bf16 only — fp32r's PE mode is
incompatible), so the four hidden-quarters' gate matmuls run CONCURRENTLY
in the four 32-column groups of the array, quadrupling effective PE
throughput for this batch-8 recurrence. Gates land at psum rows 32q, so
the eltwise cell update runs as a few [128,*] ACT/DVE instructions; the
final h=o*tanh(c) products shift partitions back to base 0 (DVE operands
may have per-operand partition bases) so the hT transposes keep the
baseline's base-0 form.

Per core (B=8 local samples, T=512 steps, tokens ordered t-major: col = t*8+b):
  P1: xT[1024, 4096] = relu(W_pre @ fT + b_pre)        (PE, fp32r)
  P2: gx[4096tok, 4096g] = xT.T @ W_ihT + (b_ih+b_hh)  (PE, fp32r; output
      rounded to bf16; gate axis host-permuted to [i_q f_q g_q o_q] per
      256-wide hidden quarter q)
  P3: 512-step LSTM recurrence (col-tiled bf16, see above)
  P4: scoresT[22, 4096] = W_cls @ hT_hist + b_cls      (bf16, batched at end)
Host reassembles [B*T, 22] from the 8 scoresT outputs.
"""
import sys

for _p in ("/opt/trn_rl_repo",):
    if _p not in sys.path:
        sys.path.insert(0, _p)

import numpy as np
import concourse.bass as bass
import concourse.mybir as mybir
import concourse.tile as tile
from concourse import bacc
from concourse.bass_utils import run_bass_kernel_spmd
from concourse.masks import make_identity

FP = mybir.dt.float32
FR = mybir.dt.float32r
BF = mybir.dt.bfloat16
AF = mybir.ActivationFunctionType

F = 3072      # feature size
LI = 1024     # LSTM input
H = 1024      # LSTM hidden
G = 4096      # 4*H gates
C = 22        # classes
B_FULL = 64
NCORES = 8
BL = B_FULL // NCORES        # 8 samples per core
P = 128
KF = F // P   # 24 k-tiles over feature
KI = LI // P  # 8 k-tiles over LSTM input
KH = H // P   # 8 k-tiles over hidden
HQ = H // 4   # 256 hidden per quarter
HIST = 16     # recurrence steps per history tile


def build_kernel(T=512, t0=0, t1=None, debug_outputs=False):
    """One program covering recurrence steps [t0, t1) of T total.

    The first program (t0==0) also runs phases 1-2 and OUTPUTS gx; later
    programs take gx plus the carried h/c state as inputs. Each program
    emits scores for its own step window."""
    if t1 is None:
        t1 = T
    TOK = BL * T
    first = (t0 == 0)
    last = (t1 == T)
    n_tok_tiles = TOK // P          # stationary token tiles in P2
    n_chunks = TOK // 512           # 512-token chunks in P1
    nc = bacc.Bacc(None, target_bir_lowering=False)

    if first:
        fT = nc.dram_tensor("fT", [F, TOK], FR, kind="ExternalInput")
        WpT = nc.dram_tensor("WpT", [F, LI], FR, kind="ExternalInput")
        bp = nc.dram_tensor("bp", [LI], FP, kind="ExternalInput")
        WiT = nc.dram_tensor("WiT", [LI, G], FR, kind="ExternalInput")
        bg = nc.dram_tensor("bg", [G], FP, kind="ExternalInput")
        gx_h = nc.dram_tensor("gx_h", [TOK, G], BF, kind="ExternalOutput")
    else:
        gx_h = nc.dram_tensor("gx_h", [TOK, G], BF, kind="ExternalInput")
        h0 = nc.dram_tensor("h0", [H, BL], BF, kind="ExternalInput")
        c0 = nc.dram_tensor("c0", [BL, H], FP, kind="ExternalInput")
    WhT = nc.dram_tensor("WhT", [H, G], BF, kind="ExternalInput")
    WcT = nc.dram_tensor("WcT", [H, C], BF, kind="ExternalInput")
    bc = nc.dram_tensor("bc", [C, 1], FP, kind="ExternalInput")
    if first:
        xT_h = nc.dram_tensor("xT_h", [LI, TOK], FR)
    hT_h = nc.dram_tensor("hT_h", [H, TOK], BF)
    sT = nc.dram_tensor("sT", [C, (t1 - t0) * BL], FP, kind="ExternalOutput")
    if not last:
        h_end = nc.dram_tensor("h_end", [H, BL], BF, kind="ExternalOutput")
        c_end = nc.dram_tensor("c_end", [BL, H], FP, kind="ExternalOutput")

    with tile.TileContext(nc) as tc:
        if first:
            # ---------------- Phase 1: xT = relu(WpT.T-tiles @ fT + bp) -------
            with (
                tc.tile_pool(name="p1w", bufs=1) as p1w,
                tc.tile_pool(name="p1f", bufs=26) as p1f,
                tc.tile_pool(name="p1o", bufs=4) as p1o,
                tc.tile_pool(name="p1b", bufs=1) as p1b,
                tc.tile_pool(name="p1ps", bufs=6, space="PSUM") as p1ps,
            ):
                wp_sb = p1w.tile([P, KF, LI], FR)
                nc.sync.dma_start(wp_sb[:], WpT[:].rearrange("(k p) m -> p k m", p=P))
                bp_sb = p1b.tile([P, KI], FP)
                nc.sync.dma_start(bp_sb[:], bp[:].rearrange("(m p) -> p m", p=P))
                for cch in range(n_chunks):
                    fts = []
                    for k in range(KF):
                        ft = p1f.tile([P, 512], FR, tag="ft")
                        nc.sync.dma_start(
                            ft[:], fT[k * P:(k + 1) * P, cch * 512:(cch + 1) * 512])
                        fts.append(ft)
                    for m in range(KI):
                        ps = p1ps.tile([P, 512], FP)
                        for k in range(KF):
                            nc.tensor.matmul(
                                ps[:], wp_sb[:, k, m * P:(m + 1) * P], fts[k][:],
                                start=(k == 0), stop=(k == KF - 1))
                        xo = p1o.tile([P, 512], FR, tag="xo")
                        nc.scalar.activation(xo[:], ps[:], AF.Relu,
                                             bias=bp_sb[:, m:m + 1])
                        nc.sync.dma_start(
                            xT_h[m * P:(m + 1) * P, cch * 512:(cch + 1) * 512], xo[:])

            # ---------------- Phase 2: gx = xT.T @ WiT + bg -------------------
            with (
                tc.tile_pool(name="p2w", bufs=1) as p2w,
                tc.tile_pool(name="p2x", bufs=3) as p2x,
                tc.tile_pool(name="p2o", bufs=4) as p2o,
                tc.tile_pool(name="p2b", bufs=1) as p2b,
                tc.tile_pool(name="p2ps", bufs=1, space="PSUM") as p2ps,
            ):
                wi_sb = p2w.tile([P, KI, G], FR)
                nc.sync.dma_start(wi_sb[:], WiT[:].rearrange("(k p) g -> p k g", p=P))
                bg_bc = p2b.tile([P, G], FP)
                nc.sync.dma_start(
                    bg_bc[:], bass.AP(tensor=bg, offset=0, ap=[[0, P], [1, G]]))
                for m in range(n_tok_tiles):
                    xs = p2x.tile([P, KI, P], FR, tag="xs")
                    nc.sync.dma_start(
                        xs[:], xT_h[:, m * P:(m + 1) * P].rearrange(
                            "(k p) n -> p k n", p=P))
                    pss = [p2ps.tile([P, 512], FP, tag=f"gps{gch}", name=f"gps{gch}_{m}") for gch in range(8)]
                    for k in range(KI):
                        for gch in range(8):
                            nc.tensor.matmul(
                                pss[gch][:], xs[:, k, :],
                                wi_sb[:, k, gch * 512:(gch + 1) * 512],
                                start=(k == 0), stop=(k == KI - 1))
                    for gch in range(8):
                        go = p2o.tile([P, 512], BF, tag="go")
                        nc.vector.tensor_tensor(
                            go[:], pss[gch][:], bg_bc[:, gch * 512:(gch + 1) * 512],
                            op=mybir.AluOpType.add)
                        nc.sync.dma_start(
                            gx_h[m * P:(m + 1) * P, gch * 512:(gch + 1) * 512], go[:])

        # ---------------- Phase 3: LSTM recurrence (col-tiled bf16) -------
        # Gate layout per step (host gate-perm gives [i_q f_q g_q o_q] per
        # 256-wide hidden quarter q at offset q*1024):
        #   P_A[32q:32q+8, 0:512]  <- chunk A_q = [i_q | f_q]
        #   P_B[32q:32q+8, 0:512]  <- chunk B_q = [g_q | o_q]
        # c lives in [128, 256] tiles: quarter q at partitions 32q:32q+8;
        # h is shifted back to [BL, H] at base 0 for the transposes.
        with (
            tc.tile_pool(name="p3w", bufs=1) as p3w,
            tc.tile_pool(name="p3gx", bufs=1) as p3gx,
            tc.tile_pool(name="p3hist", bufs=2) as p3hist,
            tc.tile_pool(name="p3st", bufs=3) as p3st,
            tc.tile_pool(name="p3tmp", bufs=2) as p3tmp,
            tc.tile_pool(name="p3one", bufs=1) as p3one,
            tc.tile_pool(name="p3pg", bufs=2, space="PSUM") as p3pg,
            tc.tile_pool(name="p3pt", bufs=2, space="PSUM") as p3pt,
            tc.tile_pool(name="p3dm", bufs=1, space="PSUM") as p3dm,
        ):
            wh_sb = p3w.tile([P, KH, G], BF)
            nc.sync.dma_start(wh_sb[:], WhT[:].rearrange("(k p) g -> p k g", p=P))

            # identities: i128 (rows 0:BL diag, bf16) for the gx psum-init
            # matmul; i8f (fp32) for the base-0 PE transposes
            i128f = p3one.tile([P, BL], FP)
            nc.gpsimd.memset(i128f[:], 0.0)
            make_identity(nc, i128f[0:BL, 0:BL], nomemset=True)
            i128 = p3one.tile([P, BL], BF)
            nc.vector.tensor_copy(i128[:], i128f[:])
            i8f = p3one.tile([BL, BL], FP)
            make_identity(nc, i8f[:])
            i8b = p3one.tile([BL, BL], BF)
            nc.vector.tensor_copy(i8b[:], i8f[:])

            if first:
                z8f = p3one.tile([P, BL], FP)
                nc.vector.memset(z8f[:], 0.0)
                z8 = p3one.tile([P, BL], BF)
                nc.vector.tensor_copy(z8[:], z8f[:])
                zc = p3one.tile([P, HQ], FP)
                nc.vector.memset(zc[:], 0.0)
                c_init = zc
            else:
                h0_sb = p3one.tile([P, KH, BL], BF)
                nc.sync.dma_start(h0_sb[:], h0[:].rearrange("(k p) b -> p k b", p=P))
                c0_sb = p3one.tile([P, HQ], FP)
                for q in range(4):
                    nc.sync.dma_start(c0_sb[32 * q:32 * q + BL, :],
                                      c0[:, q * HQ:(q + 1) * HQ])
                c_init = c0_sb

            # two manually-rotated gx staging buffers; rows BL:128 are only
            # read against the zero rows of the i128 stationary, but memset
            # them once so nothing uninitialized feeds the PE
            gxbufs = []
            for i in range(2):
                gxb = p3gx.tile([P, G], BF, name=f"gxp{i}")
                nc.vector.memset(gxb[:], 0.0)
                gxbufs.append(gxb)

            # scratch bank for keep-warm matmuls: the PE drops to its mid
            # p-state after ~3us idle and then runs the next ~4us of real
            # matmuls at half clock; a WAW-serialized stream of throwaway
            # matmuls fills the eltwise-chain window and keeps it at speed
            dm = p3dm.tile([BL, 512], FP)
            NDUMMY = 8

            hist_tiles = {}
            c_prev = c_init
            for t in range(t0, t1):
                s = t % HIST
                if s == 0 or t == t0:
                    hist = p3hist.tile([P, KH, HIST, BL], BF, tag="hist",
                                       name=f"hist_{t}")
                    hist_tiles[t // HIST] = hist

                def h_prev(k):
                    if t == t0:
                        return z8[:, 0:BL] if first else h0_sb[:, k, :]
                    ps_, pk_ = divmod(t - 1, HIST)
                    return hist_tiles[ps_][:, k, pk_, :]

                gxp = gxbufs[t % 2]
                nc.sync.dma_start(gxp[0:BL, :], gx_h[t * BL:(t + 1) * BL, :])

                pa = p3pg.tile([P, 512], FP, tag="pa")
                pb = p3pg.tile([P, 512], FP, tag="pb")
                # psum-init with gx (independent of h -> can run early)
                for q in range(4):
                    nc.tensor.matmul(pa[32 * q:32 * q + BL, :], i128[:],
                                     gxp[:, q * 1024:q * 1024 + 512],
                                     start=True, stop=False,
                                     tile_position=(0, 32 * q))
                    nc.tensor.matmul(pb[32 * q:32 * q + BL, :], i128[:],
                                     gxp[:, q * 1024 + 512:(q + 1) * 1024],
                                     start=True, stop=False,
                                     tile_position=(0, 32 * q))
                # recurrent term: 4 col groups advance together per k
                for k in range(KH):
                    for q in range(4):
                        nc.tensor.matmul(pa[32 * q:32 * q + BL, :], h_prev(k),
                                         wh_sb[:, k, q * 1024:q * 1024 + 512],
                                         start=False, stop=(k == KH - 1),
                                         tile_position=(0, 32 * q))
                        nc.tensor.matmul(pb[32 * q:32 * q + BL, :], h_prev(k),
                                         wh_sb[:, k, q * 1024 + 512:(q + 1) * 1024],
                                         start=False, stop=(k == KH - 1),
                                         tile_position=(0, 32 * q))

                for _d in range(NDUMMY):
                    nc.tensor.matmul(dm[:], i128[:], wh_sb[:, 0, 0:512],
                                     start=True, stop=True)

                # eltwise cell update, all 4 quarters per instruction; the
                # c path stays fp32, the h path drops to bf16 (2x DVE rate)
                sa = p3tmp.tile([P, 512], FP, tag="sa")    # [sig i | sig f]
                tg = p3tmp.tile([P, 512], BF, tag="tg")    # [tanh g | sig o]
                ig = p3tmp.tile([P, HQ], FP, tag="ig")
                th = p3tmp.tile([P, HQ], BF, tag="th")
                c_new = p3st.tile([P, HQ], FP, tag="c")
                h_sb = p3st.tile([BL, H], BF, tag="h")
                nc.scalar.activation(sa[:], pa[:], AF.Sigmoid)
                nc.scalar.activation(tg[:, 0:HQ], pb[:, 0:HQ], AF.Tanh)
                nc.scalar.activation(tg[:, HQ:512], pb[:, HQ:512], AF.Sigmoid)
                nc.vector.tensor_tensor(ig[:], sa[:, 0:HQ], tg[:, 0:HQ],
                                        op=mybir.AluOpType.mult)
                nc.vector.tensor_tensor(c_new[:], sa[:, HQ:512], c_prev[:],
                                        op=mybir.AluOpType.mult)
                nc.vector.tensor_tensor(c_new[:], c_new[:], ig[:],
                                        op=mybir.AluOpType.add)
                nc.scalar.activation(th[:], c_new[:], AF.Tanh)
                c_prev = c_new

                # per quarter: h = o*tanh(c) shifted to base 0, its two hT
                # transposes, and its history copy — so the k=2q,2q+1
                # matmuls of the next step unblock before later quarters
                # finish (they are issued in k order)
                pt = p3pt.tile([P, KH * BL], BF, tag="pt")
                for q in range(4):
                    nc.vector.tensor_tensor(
                        h_sb[:, q * HQ:(q + 1) * HQ],
                        tg[32 * q:32 * q + BL, HQ:512],
                        th[32 * q:32 * q + BL, :],
                        op=mybir.AluOpType.mult)
                    for k in (2 * q, 2 * q + 1):
                        nc.tensor.transpose(pt[:, k * BL:(k + 1) * BL],
                                            h_sb[:, k * P:(k + 1) * P], i8b[:])
                    nc.scalar.activation(
                        hist[:, 2 * q:2 * q + 2, s, :],
                        pt[:, 16 * q:16 * q + 16].rearrange(
                            "p (k b) -> p k b", b=BL),
                        AF.Identity)

                if s == HIST - 1:
                    tb = t - (HIST - 1)
                    for k in range(KH):
                        nc.sync.dma_start(
                            hT_h[k * P:(k + 1) * P, tb * BL:(tb + HIST) * BL],
                            hist[:, k, :, :])
                if t == t1 - 1 and not last:
                    for k in range(KH):
                        nc.sync.dma_start(h_end[k * P:(k + 1) * P, :],
                                          hist[:, k, s, :])
                    for q in range(4):
                        nc.sync.dma_start(c_end[:, q * HQ:(q + 1) * HQ],
                                          c_new[32 * q:32 * q + BL, :])

        # ---------------- Phase 4: scoresT = WcT.T-tiles @ hT + bc --------
        with (
            tc.tile_pool(name="p4w", bufs=1) as p4w,
            tc.tile_pool(name="p4h", bufs=3) as p4h,
            tc.tile_pool(name="p4o", bufs=3) as p4o,
            tc.tile_pool(name="p4ps", bufs=4, space="PSUM") as p4ps,
        ):
            wc_sb = p4w.tile([P, KH, C], BF)
            nc.sync.dma_start(wc_sb[:], WcT[:].rearrange("(k p) c -> p k c", p=P))
            bc_sb = p4w.tile([C, 1], FP)
            nc.sync.dma_start(bc_sb[:], bc[:])
            for cch in range(t0 * BL // 512, t1 * BL // 512):
                hs = p4h.tile([P, KH, 512], BF, tag="hs")
                nc.sync.dma_start(
                    hs[:], hT_h[:, cch * 512:(cch + 1) * 512].rearrange(
                        "(k p) n -> p k n", p=P))
                ps = p4ps.tile([C, 512], FP)
                for k in range(KH):
                    nc.tensor.matmul(ps[:], wc_sb[:, k, :], hs[:, k, :],
                                     start=(k == 0), stop=(k == KH - 1))
                so = p4o.tile([C, 512], FP, tag="so")
                nc.scalar.activation(so[:], ps[:], AF.Identity, bias=bc_sb[:])
                nc.sync.dma_start(
                    sT[:, cch * 512 - t0 * BL:(cch + 1) * 512 - t0 * BL], so[:])

    nc.compile()
    return nc


_NC_CACHE = {}


def _get_nc(T, t0, t1):
    key = (T, t0, t1)
    if key not in _NC_CACHE:
        _NC_CACHE[key] = build_kernel(T, t0, t1)
    return _NC_CACHE[key]


def _gate_perm():
    """Permutation of the 4096 gate axis: [i f g o] blocks of 1024 ->
    per 256-wide hidden quarter q: [i_q f_q g_q o_q]."""
    idx = []
    for q in range(4):
        for gate in range(4):
            s = gate * H + q * 256
            idx.extend(range(s, s + 256))
    return np.array(idx)


def kernel(feature_in, W_pre, b_pre, W_ih, b_ih, W_hh, b_hh, W_cls, b_cls,
           T_override=None, trace=False, n_splits=2):
    import ml_dtypes
    bf16 = ml_dtypes.bfloat16
    T = int(feature_in.shape[1]) if T_override is None else T_override
    TOK = BL * T
    perm = _gate_perm()

    WpT = np.ascontiguousarray(W_pre.T.astype(np.float32))
    WiT = np.ascontiguousarray(W_ih.T[:, perm].astype(np.float32))
    bgv = (b_ih + b_hh).astype(np.float32)[perm]
    WhT = np.ascontiguousarray(W_hh.T[:, perm].astype(bf16))
    WcT = np.ascontiguousarray(W_cls.T.astype(bf16))
    bcv = b_cls.astype(np.float32).reshape(C, 1)

    bounds = [T * i // n_splits for i in range(n_splits + 1)]
    total_ns = 0
    traces = []
    score_parts = [[] for _ in range(NCORES)]   # per core, per split
    carry = None                                # (h_end, c_end) per core
    for si in range(n_splits):
        t0, t1 = bounds[si], bounds[si + 1]
        nc = _get_nc(T, t0, t1)
        in_maps = []
        for core in range(NCORES):
            m = dict(WhT=WhT, WcT=WcT, bc=bcv)
            if t0 == 0:
                fs = feature_in[core * BL:(core + 1) * BL, :T]
                m["fT"] = np.ascontiguousarray(
                    fs.transpose(2, 1, 0).reshape(F, TOK).astype(np.float32))
                m.update(WpT=WpT, bp=b_pre.astype(np.float32), WiT=WiT, bg=bgv)
            else:
                m["gx_h"] = carry_gx[core]
                m["h0"] = carry[core][0]
                m["c0"] = carry[core][1]
            in_maps.append(m)
        res = run_bass_kernel_spmd(nc, in_maps, core_ids=list(range(NCORES)),
                                   trace=trace)
        if res.exec_time_ns is not None:
            total_ns += res.exec_time_ns
        if res.instructions_and_trace:
            traces.append(res.instructions_and_trace[1])
        for core in range(NCORES):
            r = res.results[core]
            sc = r["sT"]                                       # [22, (t1-t0)*BL]
            score_parts[core].append(sc.reshape(C, t1 - t0, BL))
        if t1 < T:
            if t0 == 0:
                carry_gx = [res.results[core]["gx_h"] for core in range(NCORES)]
            carry = [(res.results[core]["h_end"], res.results[core]["c_end"])
                     for core in range(NCORES)]

    outs = []
    for core in range(NCORES):
        sc = np.concatenate(score_parts[core], axis=1)         # [22, T, 8]
        outs.append(sc.transpose(2, 1, 0))                     # [8, T, 22]
    full = np.concatenate(outs, axis=0)                        # [64, T, 22]
    out = full.reshape(B_FULL * T, C).astype(np.float32)
    kernel.last_exec_time_ns = total_ns if total_ns else None
    kernel.last_trace = traces
    return out


kernel.last_exec_time_ns = None
kernel.last_trace = None


# revision 4
# speedup vs baseline: 1.1017x; 1.0053x over previous
"""Trainium2 Bass kernel for nn_OADModel: Linear+ReLU -> LSTM(512 steps) -> Linear.

Sharding: data-parallel over batch across 8 NeuronCores (8 samples/core).
v2: Phase 3 uses PE column tiling in bf16. A TRN2 matmul may target PSUM
partition bases {0,32,64,96} (col groups; bf16 only — fp32r's PE mode is
incompatible), so the four hidden-quarters' gate matmuls run CONCURRENTLY
in the four 32-column groups of the array, quadrupling effective PE
throughput for this batch-8 recurrence. Gates land at psum rows 32q, so
the eltwise cell update runs as a few [128,*] ACT/DVE instructions; the
final h=o*tanh(c) products shift partitions back to base 0 (DVE operands
may have per-operand partition bases) so the hT transposes keep the
baseline's base-0 form.

Per core (B=8 local samples, T=512 steps, tokens ordered t-major: col = t*8+b):
  P1: xT[1024, 4096] = relu(W_pre @ fT + b_pre)        (PE, fp32r)
  P2: gx[4096tok, 4096g] = xT.T @ W_ihT + (b_ih+b_hh)  (PE, fp32r; output
      rounded to bf16; gate axis host-permuted to [i_q f_q g_q o_q] per
      256-wide hidden quarter q)
  P3: 512-step LSTM recurrence (col-tiled bf16, see above)
  P4: scoresT[22, 4096] = W_cls @ hT_hist + b_cls      (bf16, batched at end)
Host reassembles [B*T, 22] from the 8 scoresT outputs.
"""
import sys

for _p in ("/opt/trn_rl_repo",):
    if _p not in sys.path:
        sys.path.insert(0, _p)

import numpy as np
import concourse.bass as bass
import concourse.mybir as mybir
import concourse.tile as tile
from concourse import bacc
from concourse.bass_utils import run_bass_kernel_spmd
from concourse.masks import make_identity

FP = mybir.dt.float32
FR = mybir.dt.float32r
BF = mybir.dt.bfloat16
AF = mybir.ActivationFunctionType

F = 3072      # feature size
LI = 1024     # LSTM input
H = 1024      # LSTM hidden
G = 4096      # 4*H gates
C = 22        # classes
B_FULL = 64
NCORES = 8
BL = B_FULL // NCORES        # 8 samples per core
P = 128
KF = F // P   # 24 k-tiles over feature
KI = LI // P  # 8 k-tiles over LSTM input
KH = H // P   # 8 k-tiles over hidden
HQ = H // 4   # 256 hidden per quarter
HIST = 16     # recurrence steps per history tile


def build_kernel(T=512, t0=0, t1=None, debug_outputs=False):
    """One program covering recurrence steps [t0, t1) of T total.

    The first program (t0==0) also runs phases 1-2 and OUTPUTS gx; later
    programs take gx plus the carried h/c state as inputs. Each program
    emits scores for its own step window."""
    if t1 is None:
        t1 = T
    TOK = BL * T
    first = (t0 == 0)
    last = (t1 == T)
    n_tok_tiles = TOK // P          # stationary token tiles in P2
    n_chunks = TOK // 512           # 512-token chunks in P1
    nc = bacc.Bacc(None, target_bir_lowering=False)

    if first:
        fT = nc.dram_tensor("fT", [F, TOK], FR, kind="ExternalInput")
        WpT = nc.dram_tensor("WpT", [F, LI], FR, kind="ExternalInput")
        bp = nc.dram_tensor("bp", [LI], FP, kind="ExternalInput")
        WiT = nc.dram_tensor("WiT", [LI, G], FR, kind="ExternalInput")
        bg = nc.dram_tensor("bg", [G], FP, kind="ExternalInput")
        gx_h = nc.dram_tensor("gx_h", [TOK, G], BF, kind="ExternalOutput")
    else:
        gx_h = nc.dram_tensor("gx_h", [TOK, G], BF, kind="ExternalInput")
        h0 = nc.dram_tensor("h0", [H, BL], BF, kind="ExternalInput")
        c0 = nc.dram_tensor("c0", [BL, H], BF, kind="ExternalInput")
    WhT = nc.dram_tensor("WhT", [H, G], BF, kind="ExternalInput")
    WcT = nc.dram_tensor("WcT", [H, C], BF, kind="ExternalInput")
    bc = nc.dram_tensor("bc", [C, 1], FP, kind="ExternalInput")
    if first:
        xT_h = nc.dram_tensor("xT_h", [LI, TOK], FR)
    hT_h = nc.dram_tensor("hT_h", [H, TOK], BF)
    sT = nc.dram_tensor("sT", [C, (t1 - t0) * BL], FP, kind="ExternalOutput")
    if not last:
        h_end = nc.dram_tensor("h_end", [H, BL], BF, kind="ExternalOutput")
        c_end = nc.dram_tensor("c_end", [BL, H], BF, kind="ExternalOutput")

    with tile.TileContext(nc) as tc:
        if first:
            # ---------------- Phase 1: xT = relu(WpT.T-tiles @ fT + bp) -------
            with (
                tc.tile_pool(name="p1w", bufs=1) as p1w,
                tc.tile_pool(name="p1f", bufs=26) as p1f,
                tc.tile_pool(name="p1o", bufs=4) as p1o,
                tc.tile_pool(name="p1b", bufs=1) as p1b,
                tc.tile_pool(name="p1ps", bufs=6, space="PSUM") as p1ps,
            ):
                wp_sb = p1w.tile([P, KF, LI], FR)
                nc.sync.dma_start(wp_sb[:], WpT[:].rearrange("(k p) m -> p k m", p=P))
                bp_sb = p1b.tile([P, KI], FP)
                nc.sync.dma_start(bp_sb[:], bp[:].rearrange("(m p) -> p m", p=P))
                for cch in range(n_chunks):
                    fts = []
                    for k in range(KF):
                        ft = p1f.tile([P, 512], FR, tag="ft")
                        nc.sync.dma_start(
                            ft[:], fT[k * P:(k + 1) * P, cch * 512:(cch + 1) * 512])
                        fts.append(ft)
                    for m in range(KI):
                        ps = p1ps.tile([P, 512], FP)
                        for k in range(KF):
                            nc.tensor.matmul(
                                ps[:], wp_sb[:, k, m * P:(m + 1) * P], fts[k][:],
                                start=(k == 0), stop=(k == KF - 1))
                        xo = p1o.tile([P, 512], FR, tag="xo")
                        nc.scalar.activation(xo[:], ps[:], AF.Relu,
                                             bias=bp_sb[:, m:m + 1])
                        nc.sync.dma_start(
                            xT_h[m * P:(m + 1) * P, cch * 512:(cch + 1) * 512], xo[:])

            # ---------------- Phase 2: gx = xT.T @ WiT + bg -------------------
            with (
                tc.tile_pool(name="p2w", bufs=1) as p2w,
                tc.tile_pool(name="p2x", bufs=3) as p2x,
                tc.tile_pool(name="p2o", bufs=4) as p2o,
                tc.tile_pool(name="p2b", bufs=1) as p2b,
                tc.tile_pool(name="p2ps", bufs=1, space="PSUM") as p2ps,
            ):
                wi_sb = p2w.tile([P, KI, G], FR)
                nc.sync.dma_start(wi_sb[:], WiT[:].rearrange("(k p) g -> p k g", p=P))
                bg_bc = p2b.tile([P, G], FP)
                nc.sync.dma_start(
                    bg_bc[:], bass.AP(tensor=bg, offset=0, ap=[[0, P], [1, G]]))
                for m in range(n_tok_tiles):
                    xs = p2x.tile([P, KI, P], FR, tag="xs")
                    nc.sync.dma_start(
                        xs[:], xT_h[:, m * P:(m + 1) * P].rearrange(
                            "(k p) n -> p k n", p=P))
                    pss = [p2ps.tile([P, 512], FP, tag=f"gps{gch}", name=f"gps{gch}_{m}") for gch in range(8)]
                    for k in range(KI):
                        for gch in range(8):
                            nc.tensor.matmul(
                                pss[gch][:], xs[:, k, :],
                                wi_sb[:, k, gch * 512:(gch + 1) * 512],
                                start=(k == 0), stop=(k == KI - 1))
                    for gch in range(8):
                        go = p2o.tile([P, 512], BF, tag="go")
                        nc.vector.tensor_tensor(
                            go[:], pss[gch][:], bg_bc[:, gch * 512:(gch + 1) * 512],
                            op=mybir.AluOpType.add)
                        nc.sync.dma_start(
                            gx_h[m * P:(m + 1) * P, gch * 512:(gch + 1) * 512], go[:])

        # ---------------- Phase 3: LSTM recurrence (col-tiled bf16) -------
        # Gate layout per step (host gate-perm gives [i_q f_q g_q o_q] per
        # 256-wide hidden quarter q at offset q*1024):
        #   P_A[32q:32q+8, 0:512]  <- chunk A_q = [i_q | f_q]
        #   P_B[32q:32q+8, 0:512]  <- chunk B_q = [g_q | o_q]
        # c lives in [128, 256] tiles: quarter q at partitions 32q:32q+8;
        # h is shifted back to [BL, H] at base 0 for the transposes.
        with (
            tc.tile_pool(name="p3w", bufs=1) as p3w,
            tc.tile_pool(name="p3gx", bufs=1) as p3gx,
            tc.tile_pool(name="p3hist", bufs=2) as p3hist,
            tc.tile_pool(name="p3st", bufs=3) as p3st,
            tc.tile_pool(name="p3tmp", bufs=2) as p3tmp,
            tc.tile_pool(name="p3one", bufs=1) as p3one,
            tc.tile_pool(name="p3pg", bufs=2, space="PSUM") as p3pg,
            tc.tile_pool(name="p3pt", bufs=2, space="PSUM") as p3pt,
            tc.tile_pool(name="p3dm", bufs=1, space="PSUM") as p3dm,
        ):
            wh_sb = p3w.tile([P, KH, G], BF)
            nc.sync.dma_start(wh_sb[:], WhT[:].rearrange("(k p) g -> p k g", p=P))

            # identities: i128 (rows 0:BL diag, bf16) for the gx psum-init
            # matmul; i8f (fp32) for the base-0 PE transposes
            i128f = p3one.tile([P, BL], FP)
            nc.gpsimd.memset(i128f[:], 0.0)
            make_identity(nc, i128f[0:BL, 0:BL], nomemset=True)
            i128 = p3one.tile([P, BL], BF)
            nc.vector.tensor_copy(i128[:], i128f[:])
            i8f = p3one.tile([BL, BL], FP)
            make_identity(nc, i8f[:])
            i8b = p3one.tile([BL, BL], BF)
            nc.vector.tensor_copy(i8b[:], i8f[:])

            if first:
                z8f = p3one.tile([P, BL], FP)
                nc.vector.memset(z8f[:], 0.0)
                z8 = p3one.tile([P, BL], BF)
                nc.vector.tensor_copy(z8[:], z8f[:])
                zc = p3one.tile([P, HQ], BF)
                nc.vector.memset(zc[:], 0.0)
                c_init = zc
            else:
                h0_sb = p3one.tile([P, KH, BL], BF)
                nc.sync.dma_start(h0_sb[:], h0[:].rearrange("(k p) b -> p k b", p=P))
                c0_sb = p3one.tile([P, HQ], BF)
                for q in range(4):
                    nc.sync.dma_start(c0_sb[32 * q:32 * q + BL, :],
                                      c0[:, q * HQ:(q + 1) * HQ])
                c_init = c0_sb

            # two manually-rotated gx staging buffers; rows BL:128 are only
            # read against the zero rows of the i128 stationary, but memset
            # them once so nothing uninitialized feeds the PE
            gxbufs = []
            for i in range(2):
                gxb = p3gx.tile([P, G], BF, name=f"gxp{i}")
                nc.vector.memset(gxb[:], 0.0)
                gxbufs.append(gxb)

            # scratch bank for keep-warm matmuls: the PE drops to its mid
            # p-state after ~3us idle and then runs the next ~4us of real
            # matmuls at half clock; a WAW-serialized stream of throwaway
            # matmuls fills the eltwise-chain window and keeps it at speed
            dm = p3dm.tile([BL, 512], FP)
            NDUMMY = 16          # N=256 each: finer quanta block real
                                 # matmuls for at most ~110ns apiece

            hist_tiles = {}
            c_prev = c_init
            for t in range(t0, t1):
                s = t % HIST
                if s == 0 or t == t0:
                    hist = p3hist.tile([P, KH, HIST, BL], BF, tag="hist",
                                       name=f"hist_{t}")
                    hist_tiles[t // HIST] = hist

                def h_prev(k):
                    if t == t0:
                        return z8[:, 0:BL] if first else h0_sb[:, k, :]
                    ps_, pk_ = divmod(t - 1, HIST)
                    return hist_tiles[ps_][:, k, pk_, :]

                gxp = gxbufs[t % 2]
                nc.sync.dma_start(gxp[0:BL, :], gx_h[t * BL:(t + 1) * BL, :])

                pa = p3pg.tile([P, 512], FP, tag="pa")
                pb = p3pg.tile([P, 512], FP, tag="pb")
                # psum-init with gx (independent of h -> can run early)
                for q in range(4):
                    nc.tensor.matmul(pa[32 * q:32 * q + BL, :], i128[:],
                                     gxp[:, q * 1024:q * 1024 + 512],
                                     start=True, stop=False,
                                     tile_position=(0, 32 * q))
                    nc.tensor.matmul(pb[32 * q:32 * q + BL, :], i128[:],
                                     gxp[:, q * 1024 + 512:(q + 1) * 1024],
                                     start=True, stop=False,
                                     tile_position=(0, 32 * q))
                # recurrent term: 4 col groups advance together per k
                for k in range(KH):
                    for q in range(4):
                        nc.tensor.matmul(pa[32 * q:32 * q + BL, :], h_prev(k),
                                         wh_sb[:, k, q * 1024:q * 1024 + 512],
                                         start=False, stop=(k == KH - 1),
                                         tile_position=(0, 32 * q))
                        nc.tensor.matmul(pb[32 * q:32 * q + BL, :], h_prev(k),
                                         wh_sb[:, k, q * 1024 + 512:(q + 1) * 1024],
                                         start=False, stop=(k == KH - 1),
                                         tile_position=(0, 32 * q))

                for _d in range(NDUMMY):
                    nc.tensor.matmul(dm[:, 0:256], i128[:], wh_sb[:, 0, 0:256],
                                     start=True, stop=True)

                # eltwise cell update, all 4 quarters per instruction, all
                # bf16 (16-bit runs at 2x DVE rate; c's bf16 rounding decays
                # through the forget gate instead of accumulating)
                sa = p3tmp.tile([P, 512], BF, tag="sa")    # [sig i | sig f]
                tg = p3tmp.tile([P, 512], BF, tag="tg")    # [tanh g | sig o]
                ig = p3tmp.tile([P, HQ], BF, tag="ig")
                th = p3tmp.tile([P, HQ], BF, tag="th")
                c_new = p3st.tile([P, HQ], BF, tag="c")
                h_sb = p3st.tile([BL, H], BF, tag="h")
                nc.scalar.activation(sa[:], pa[:], AF.Sigmoid)
                nc.scalar.activation(tg[:, 0:HQ], pb[:, 0:HQ], AF.Tanh)
                nc.scalar.activation(tg[:, HQ:512], pb[:, HQ:512], AF.Sigmoid)
                nc.vector.tensor_tensor(ig[:], sa[:, 0:HQ], tg[:, 0:HQ],
                                        op=mybir.AluOpType.mult)
                nc.vector.tensor_tensor(c_new[:], sa[:, HQ:512], c_prev[:],
                                        op=mybir.AluOpType.mult)
                nc.vector.tensor_tensor(c_new[:], c_new[:], ig[:],
                                        op=mybir.AluOpType.add)
                nc.scalar.activation(th[:], c_new[:], AF.Tanh)
                c_prev = c_new

                # per quarter: h = o*tanh(c) shifted to base 0, its two hT
                # transposes, and its history copy — so the k=2q,2q+1
                # matmuls of the next step unblock before later quarters
                # finish (they are issued in k order)
                pt = p3pt.tile([P, KH * BL], BF, tag="pt")
                for q in range(4):
                    nc.vector.tensor_tensor(
                        h_sb[:, q * HQ:(q + 1) * HQ],
                        tg[32 * q:32 * q + BL, HQ:512],
                        th[32 * q:32 * q + BL, :],
                        op=mybir.AluOpType.mult)
                    for k in (2 * q, 2 * q + 1):
                        nc.tensor.transpose(pt[:, k * BL:(k + 1) * BL],
                                            h_sb[:, k * P:(k + 1) * P], i8b[:])
                    nc.scalar.activation(
                        hist[:, 2 * q:2 * q + 2, s, :],
                        pt[:, 16 * q:16 * q + 16].rearrange(
                            "p (k b) -> p k b", b=BL),
                        AF.Identity)

                if s == HIST - 1:
                    tb = t - (HIST - 1)
                    for k in range(KH):
                        nc.sync.dma_start(
                            hT_h[k * P:(k + 1) * P, tb * BL:(tb + HIST) * BL],
                            hist[:, k, :, :])
                if t == t1 - 1 and not last:
                    for k in range(KH):
                        nc.sync.dma_start(h_end[k * P:(k + 1) * P, :],
                                          hist[:, k, s, :])
                    for q in range(4):
                        nc.sync.dma_start(c_end[:, q * HQ:(q + 1) * HQ],
                                          c_new[32 * q:32 * q + BL, :])

        # ---------------- Phase 4: scoresT = WcT.T-tiles @ hT + bc --------
        with (
            tc.tile_pool(name="p4w", bufs=1) as p4w,
            tc.tile_pool(name="p4h", bufs=3) as p4h,
            tc.tile_pool(name="p4o", bufs=3) as p4o,
            tc.tile_pool(name="p4ps", bufs=4, space="PSUM") as p4ps,
        ):
            wc_sb = p4w.tile([P, KH, C], BF)
            nc.sync.dma_start(wc_sb[:], WcT[:].rearrange("(k p) c -> p k c", p=P))
            bc_sb = p4w.tile([C, 1], FP)
            nc.sync.dma_start(bc_sb[:], bc[:])
            for cch in range(t0 * BL // 512, t1 * BL // 512):
                hs = p4h.tile([P, KH, 512], BF, tag="hs")
                nc.sync.dma_start(
                    hs[:], hT_h[:, cch * 512:(cch + 1) * 512].rearrange(
                        "(k p) n -> p k n", p=P))
                ps = p4ps.tile([C, 512], FP)
                for k in range(KH):
                    nc.tensor.matmul(ps[:], wc_sb[:, k, :], hs[:, k, :],
                                     start=(k == 0), stop=(k == KH - 1))
                so = p4o.tile([C, 512], FP, tag="so")
                nc.scalar.activation(so[:], ps[:], AF.Identity, bias=bc_sb[:])
                nc.sync.dma_start(
                    sT[:, cch * 512 - t0 * BL:(cch + 1) * 512 - t0 * BL], so[:])

    nc.compile()
    return nc


_NC_CACHE = {}


def _get_nc(T, t0, t1):
    key = (T, t0, t1)
    if key not in _NC_CACHE:
        _NC_CACHE[key] = build_kernel(T, t0, t1)
    return _NC_CACHE[key]


def _gate_perm():
    """Permutation of the 4096 gate axis: [i f g o] blocks of 1024 ->
    per 256-wide hidden quarter q: [i_q f_q g_q o_q]."""
    idx = []
    for q in range(4):
        for gate in range(4):
            s = gate * H + q * 256
            idx.extend(range(s, s + 256))
    return np.array(idx)


def kernel(feature_in, W_pre, b_pre, W_ih, b_ih, W_hh, b_hh, W_cls, b_cls,
           T_override=None, trace=False, n_splits=2):
    import ml_dtypes
    bf16 = ml_dtypes.bfloat16
    T = int(feature_in.shape[1]) if T_override is None else T_override
    TOK = BL * T
    perm = _gate_perm()

    WpT = np.ascontiguousarray(W_pre.T.astype(np.float32))
    WiT = np.ascontiguousarray(W_ih.T[:, perm].astype(np.float32))
    bgv = (b_ih + b_hh).astype(np.float32)[perm]
    WhT = np.ascontiguousarray(W_hh.T[:, perm].astype(bf16))
    WcT = np.ascontiguousarray(W_cls.T.astype(bf16))
    bcv = b_cls.astype(np.float32).reshape(C, 1)

    bounds = [T * i // n_splits for i in range(n_splits + 1)]
    total_ns = 0
    traces = []
    score_parts = [[] for _ in range(NCORES)]   # per core, per split
    carry = None                                # (h_end, c_end) per core
    for si in range(n_splits):
        t0, t1 = bounds[si], bounds[si + 1]
        nc = _get_nc(T, t0, t1)
        in_maps = []
        for core in range(NCORES):
            m = dict(WhT=WhT, WcT=WcT, bc=bcv)
            if t0 == 0:
                fs = feature_in[core * BL:(core + 1) * BL, :T]
                m["fT"] = np.ascontiguousarray(
                    fs.transpose(2, 1, 0).reshape(F, TOK).astype(np.float32))
                m.update(WpT=WpT, bp=b_pre.astype(np.float32), WiT=WiT, bg=bgv)
            else:
                m["gx_h"] = carry_gx[core]
                m["h0"] = carry[core][0]
                m["c0"] = carry[core][1]
            in_maps.append(m)
        res = run_bass_kernel_spmd(nc, in_maps, core_ids=list(range(NCORES)),
                                   trace=trace)
        if res.exec_time_ns is not None:
            total_ns += res.exec_time_ns
        if res.instructions_and_trace:
            traces.append(res.instructions_and_trace[1])
        for core in range(NCORES):
            r = res.results[core]
            sc = r["sT"]                                       # [22, (t1-t0)*BL]
            score_parts[core].append(sc.reshape(C, t1 - t0, BL))
        if t1 < T:
            if t0 == 0:
                carry_gx = [res.results[core]["gx_h"] for core in range(NCORES)]
            carry = [(res.results[core]["h_end"], res.results[core]["c_end"])
                     for core in range(NCORES)]

    outs = []
    for core in range(NCORES):
        sc = np.concatenate(score_parts[core], axis=1)         # [22, T, 8]
        outs.append(sc.transpose(2, 1, 0))                     # [8, T, 22]
    full = np.concatenate(outs, axis=0)                        # [64, T, 22]
    out = full.reshape(B_FULL * T, C).astype(np.float32)
    kernel.last_exec_time_ns = total_ns if total_ns else None
    kernel.last_trace = traces
    return out


kernel.last_exec_time_ns = None
kernel.last_trace = None
